# revision 1
# baseline (speedup 1.0000x reference)
"""AGNNConv distributed Bass kernel for 8 TRN2 NeuronCores (v4).

out = (1+eps)*feat + h,  h[d] = sum_{e: dst_e=d} p_e * norm_feat[src_e]
with p_e = edge-softmax grouped by src.

Algebra (softmax max-subtraction dropped -- identity in exact math):
    w_e = exp(beta*ew_e)
    z_n = sum_{e: src_e=n} w_e
    q_e = w_e / (||feat_src_e|| * z_src_e)     # per-edge scalar
    h_d = sum_{e: dst_e=d} q_e * feat[src_e]
    out = (1+eps)*feat + h

The per-edge gather of source features is a pure permutation of input rows
-> done on HOST (no float math). Device receives contiguous per-edge streams:
  feat_edges [128, tet*D] f32   feat[src_e] rows in edge-slot order
  zpadE      [128, tet*K] f32   src's K-slot padded edge-weight row per edge
  ewp        [128, tet]   f32   this edge's weight
  bitp       [14, tet*128] bf16 dst-within-tile bit planes (one-hot trick)

v5 engine layout (v3 was Vector-bound; v4's 2-byte strided-DMA cast blew up
descriptor count):
  - msg stays FP32 end to end; mm2 runs fp32 (4 cyc/row, LDWEIGHTS hidden
    under the 213ns streams) -- no f32->bf16 cast pass exists at all.
  - q folded into msg rows on GpSimd (677 small 1-input tensor_scalar_mul).
  - one-hot st built 4 tiles per op from a full PSUM bank (plain is_eq /
    relu(M-6), no per-tile scale), alternating Vector 2 : Scalar 1.
  - 1/sqrt(ss) as exp(-0.5*ln(ss)) so every ScalarE ACT (Exp/Ln/Copy/Relu/
    Square) stays in one table set -- v3 paid 76 ACT_TABLE_LOADs (97us).
  - mm1 batches run one 4-batch ahead of mm2 so the PE never stalls on the
    st round-trip.
"""

import sys

sys.path.insert(0, "/opt/trn_rl_repo")

import numpy as np

N, E, D = 50000, 640000, 128
NCORES = 8
SH = N // NCORES            # 6250 dst nodes per core
HTILES = (SH + 127) // 128  # 49 dst tiles per core

PAD_EW = -80.0              # exp(beta*PAD_EW) == 0 (inside ACT LUT range)


def _host_prep(src, dst, edge_weight):
    """Index/layout prep only (no float math on tensor values)."""
    src = np.asarray(src).astype(np.int64)
    dst = np.asarray(dst).astype(np.int64)
    ew = np.asarray(edge_weight).astype(np.float32)

    # ---- per-node src-grouped edge-weight rows (for z), fixed K ----
    deg = np.bincount(src, minlength=N)
    K = int(deg.max())
    order = np.argsort(src, kind="stable")
    starts = np.zeros(N + 1, np.int64)
    np.cumsum(deg, out=starts[1:])
    slot = np.arange(E, dtype=np.int64) - starts[src[order]]
    zpad = np.full((N + 1, K), PAD_EW, np.float32)
    zpad[src[order], slot] = ew[order]
    # pad node (index N): slot0 = 0 -> z = 1 for pad edges (q ~ 0 safely)
    zpad[N, 0] = 0.0

    # ---- per-core edge grouping by dst tile ----
    owner = dst // SH
    dstl = dst - owner * SH
    dtile = dstl // 128
    dbit = dstl % 128

    counts = np.zeros((NCORES, HTILES), np.int64)
    np.add.at(counts, (owner, dtile), 1)
    net = (counts.max(axis=0) + 127) // 128  # [HTILES] edge tiles per dst tile
    net = np.maximum(net, 1)
    tet = int(net.sum())
    seg_off = np.zeros(HTILES, np.int64)
    seg_off[1:] = np.cumsum(net)[:-1]
    EPAD = tet * 128

    core_idx = []
    for c in range(NCORES):
        m = np.nonzero(owner == c)[0]
        key = dtile[m]
        korder = np.argsort(key, kind="stable")
        me = m[korder]
        keys = key[korder]
        kb = np.r_[0, np.nonzero(np.diff(keys))[0] + 1]
        sf = np.zeros(len(keys), np.int64)
        sf[kb] = kb
        np.maximum.accumulate(sf, out=sf)
        within = np.arange(len(keys)) - sf
        pos = seg_off[keys] * 128 + within

        src_pad = np.full(EPAD, N, np.int64)      # pad edges read node N
        ewp = np.full(EPAD, PAD_EW, np.float32)
        bits = np.zeros(EPAD, np.int64)
        src_pad[pos] = src[me]
        ewp[pos] = ew[me]
        bits[pos] = dbit[me]

        bp = np.zeros((14, EPAD), np.float32)
        for b in range(7):
            bb = (bits >> b) & 1
            bp[2 * b + 1] = bb
            bp[2 * b] = 1 - bb
        core_idx.append((src_pad, ewp.reshape(tet, 128).T.copy(), bp))

    import ml_dtypes

    nb = np.zeros((14, 128), np.float32)
    nn = np.arange(128)
    for b in range(7):
        bb = (nn >> b) & 1
        nb[2 * b + 1] = bb
        nb[2 * b] = 1 - bb
    nb = nb.astype(ml_dtypes.bfloat16)

    return zpad, core_idx, nb, net, K


_COMPILED = {}


def _build(net, K):
    import concourse.bass as bass
    import concourse.bacc as bacc
    from concourse import mybir, tile

    f32 = mybir.dt.float32
    bf16 = mybir.dt.bfloat16
    u16 = mybir.dt.uint16
    AF = mybir.ActivationFunctionType
    ALU = mybir.AluOpType
    X = mybir.AxisListType.X

    tet = int(net.sum())
    nmax = int(net.max())

    nc = bacc.Bacc(None, debug=False)

    fe_ext = nc.dram_tensor("feat_edges", [128, tet * D], f32, kind="ExternalInput")
    ze_ext = nc.dram_tensor("zpadE", [128, tet * K], f32, kind="ExternalInput")
    ewp_ext = nc.dram_tensor("ewp", [128, tet], f32, kind="ExternalInput")
    bitp_ext = nc.dram_tensor("bitp", [14, tet * 128], bf16, kind="ExternalInput")
    nbits_ext = nc.dram_tensor("nbits", [14, 128], bf16, kind="ExternalInput")
    featmy_ext = nc.dram_tensor("feat_my", [SH, D], f32, kind="ExternalInput")
    beta_ext = nc.dram_tensor("beta", [1, 1], f32, kind="ExternalInput")
    eps_ext = nc.dram_tensor("eps", [1, 1], f32, kind="ExternalInput")
    out_ext = nc.dram_tensor("out", [SH, D], f32, kind="ExternalOutput")

    GRP = 4          # dst tiles per norm group (ACT table loads amortized)
    ZCH = 96         # z pre-phase: edge tiles per chunk

    with tile.TileContext(nc) as tc:
        with (
            tc.tile_pool(name="persist", bufs=1) as pp,
            tc.tile_pool(name="msgp", bufs=GRP + 2) as mpool,
            tc.tile_pool(name="msgq", bufs=3) as mqpool,
            tc.tile_pool(name="sqp", bufs=3) as qpool,
            tc.tile_pool(name="zpre", bufs=2) as zpool,
            tc.tile_pool(name="small", bufs=2 * GRP + 2) as spool,
            tc.tile_pool(name="bitpool", bufs=GRP + 2) as bpool,
            tc.tile_pool(name="stp", bufs=4) as stpool,
            tc.tile_pool(name="outp", bufs=4) as opool,
            tc.tile_pool(name="mpsum", bufs=4, space="PSUM") as mpsum,
            tc.tile_pool(name="hpsum", bufs=2, space="PSUM") as hpsum,
        ):
            # ---------- scalars ----------
            beta_s = pp.tile([1, 1], f32, tag="beta_s")
            eps_s = pp.tile([1, 1], f32, tag="eps_s")
            nc.sync.dma_start(out=beta_s[:], in_=beta_ext[:])
            nc.sync.dma_start(out=eps_s[:], in_=eps_ext[:])
            beta_b = pp.tile([128, 1], f32, tag="beta_b")
            ep1_b = pp.tile([128, 1], f32, tag="ep1_b")
            nc.gpsimd.partition_broadcast(beta_b[:], beta_s[:])
            nc.gpsimd.partition_broadcast(ep1_b[:], eps_s[:])
            nc.vector.tensor_scalar_add(ep1_b[:], ep1_b[:], 1.0)
            neg6 = pp.tile([128, 1], f32, tag="neg6")
            nc.vector.memset(neg6[:], -6.0)
            sseps = pp.tile([128, 1], f32, tag="sseps")
            nc.vector.memset(sseps[:], 1e-12)

            # ---------- global per-edge weight w = exp(beta*ew) ----------
            wv = pp.tile([128, tet], f32, tag="wv")
            nc.sync.dma_start(out=wv[:], in_=ewp_ext[:])
            nc.scalar.activation(wv[:], wv[:], AF.Exp, scale=beta_b[:])

            nbits = pp.tile([14, 128], bf16, tag="nbits")
            nc.sync.dma_start(out=nbits[:], in_=nbits_ext[:])

            # ---------- z pre-phase: z = sum exp(beta*zrow), wz = w/z ------
            zs = pp.tile([128, tet], f32, tag="zs")
            for c0 in range(0, tet, ZCH):
                cw = min(ZCH, tet - c0)
                zrow = zpool.tile([128, ZCH, K], f32, tag="zrow")
                nc.sync.dma_start(
                    out=zrow[:, :cw, :].rearrange("p a b -> p (a b)"),
                    in_=ze_ext[:, c0 * K : (c0 + cw) * K],
                )
                zx = zpool.tile([128, ZCH, K], bf16, tag="zx")
                nc.scalar.activation(
                    zx[:, :cw, :].rearrange("p a b -> p (a b)"),
                    zrow[:, :cw, :].rearrange("p a b -> p (a b)"),
                    AF.Exp,
                    scale=beta_b[:],
                )
                nc.vector.tensor_reduce(
                    zs[:, c0 : c0 + cw], zx[:, :cw, :], X, ALU.add
                )
            wz = pp.tile([128, tet], f32, tag="wz")
            nc.vector.reciprocal(wz[:], zs[:])
            nc.vector.tensor_tensor(wz[:], wz[:], wv[:], ALU.mult)

            # ---------- main loop: groups of GRP dst tiles ----------
            offs = [0] * (HTILES + 1)
            for i in range(HTILES):
                offs[i + 1] = offs[i] + int(net[i])

            for i0 in range(0, HTILES, GRP):
                tiles = list(range(i0, min(i0 + GRP, HTILES)))
                msgs, sqs, sss, lnss, rrs, qs, bps = {}, {}, {}, {}, {}, {}, {}

                for i in tiles:
                    nh, T = int(net[i]), offs[i]
                    msg = mpool.tile([128, nmax, D], f32, tag="msg")
                    nc.sync.dma_start(
                        out=msg[:, :nh, :].rearrange("p a b -> p (a b)"),
                        in_=fe_ext[:, T * D : (T + nh) * D],
                    )
                    msgs[i] = msg
                    bp = bpool.tile([14, nmax, 128], bf16, tag="bp")
                    nc.sync.dma_start(
                        out=bp[:, :nh, :].rearrange("p a b -> p (a b)"),
                        in_=bitp_ext[:, T * 128 : (T + nh) * 128],
                    )
                    bps[i] = bp
                    sq = qpool.tile([128, nmax, D], bf16, tag="sq")
                    nc.gpsimd.tensor_tensor(
                        sq[:, :nh, :].rearrange("p a b -> p (a b)"),
                        msg[:, :nh, :].rearrange("p a b -> p (a b)"),
                        msg[:, :nh, :].rearrange("p a b -> p (a b)"),
                        ALU.mult,
                    )
                    sqs[i] = sq
                    ss = spool.tile([128, nmax], f32, tag="ss")
                    nc.vector.tensor_reduce(ss[:, :nh], sq[:, :nh, :], X, ALU.add)
                    sss[i] = ss

                # rr = 1/sqrt(ss) via int bit-hack + one Newton step (Vector
                # only -- keeps Ln/Exp off ScalarE so the main loop needs no
                # ACT table switches). q = rr * w/z.
                for i in tiles:
                    nh, T = int(net[i]), offs[i]
                    ss = sss[i]
                    i32 = mybir.dt.int32
                    y0 = spool.tile([128, nmax], f32, tag="y0")
                    nc.vector.tensor_scalar(
                        y0[:, :nh].bitcast(i32), ss[:, :nh].bitcast(i32),
                        1, -1, op0=ALU.arith_shift_right, op1=ALU.bitwise_xor,
                    )
                    nc.vector.tensor_scalar(
                        y0[:, :nh].bitcast(i32), y0[:, :nh].bitcast(i32),
                        0x5F3759E0, None, op0=ALU.add,
                    )
                    u = spool.tile([128, nmax], f32, tag="u")
                    nc.vector.tensor_tensor(u[:, :nh], y0[:, :nh], y0[:, :nh], ALU.mult)
                    nc.vector.tensor_tensor(u[:, :nh], u[:, :nh], ss[:, :nh], ALU.mult)
                    nc.vector.tensor_scalar(
                        u[:, :nh], u[:, :nh], -0.5, 1.5, op0=ALU.mult, op1=ALU.add
                    )
                    nc.vector.tensor_tensor(y0[:, :nh], y0[:, :nh], u[:, :nh], ALU.mult)
                    q = spool.tile([128, nmax], f32, tag="q")
                    nc.vector.tensor_tensor(
                        q[:, :nh], y0[:, :nh], wz[:, T : T + nh], ALU.mult
                    )
                    qs[i] = q

                for i in tiles:
                    nh, T = int(net[i]), offs[i]
                    nb4 = (nh + 3) // 4
                    rows = min(128, SH - i * 128)
                    msg, bp, q = msgs[i], bps[i], qs[i]

                    # q-scaled bf16 message rows (q broadcast along D)
                    msgq = mqpool.tile([128, nmax, D], bf16, tag="msgq")
                    qb = q[:, :nh].unsqueeze(2).broadcast_to([128, nh, D])
                    nc.vector.tensor_tensor(
                        msgq[:, :nh, :], msg[:, :nh, :], qb, ALU.mult
                    )

                    # edge tiles: batched one-hot build, mm1 one batch ahead
                    hp = hpsum.tile([128, D], f32, tag="hp")
                    st4s = [None] * nb4

                    def issue_batch(g):
                        t0 = 4 * g
                        w4 = min(4, nh - t0)
                        mp4 = mpsum.tile([128, 4, 128], f32, tag="mp4")
                        for j in range(w4):
                            nc.tensor.matmul(
                                mp4[:, j, :], bp[:, t0 + j, :], nbits[:],
                                start=True, stop=True,
                            )
                        st4 = stpool.tile([128, 4, 128], bf16, tag="st4")
                        sflat = st4[:, :w4, :].rearrange("p a b -> p (a b)")
                        mflat = mp4[:, :w4, :].rearrange("p a b -> p (a b)")
                        nc.scalar.activation(
                            sflat, mflat, AF.Relu, bias=neg6[:]
                        )
                        st4s[g] = st4

                    for g in range(nb4 + 1):
                        if g < nb4:
                            issue_batch(g)
                        gm = g - 1
                        if gm >= 0:
                            t0 = 4 * gm
                            for j in range(min(4, nh - t0)):
                                t = t0 + j
                                nc.tensor.matmul(
                                    hp[:],
                                    st4s[gm][:, j, :],
                                    msgq[:, t, :],
                                    start=(t == 0),
                                    stop=(t == nh - 1),
                                )

                    # out = hp + (1+eps)*feat_my
                    ftm = opool.tile([128, D], f32, tag="ftm")
                    nc.sync.dma_start(
                        out=ftm[:rows, :],
                        in_=featmy_ext[i * 128 : i * 128 + rows, :],
                    )
                    fts = opool.tile([128, D], f32, tag="fts")
                    nc.scalar.activation(
                        fts[:rows, :], ftm[:rows, :], AF.Copy,
                        scale=ep1_b[:rows, :],
                    )
                    ot = opool.tile([128, D], f32, tag="ot")
                    nc.vector.tensor_tensor(
                        ot[:rows, :], fts[:rows, :], hp[:rows, :], ALU.add
                    )
                    nc.sync.dma_start(
                        out=out_ext[i * 128 : i * 128 + rows, :], in_=ot[:rows, :]
                    )

    nc.finalize()
    return nc


def kernel(feat, edge_weight, src, dst, beta, eps):
    from concourse.bass_utils import run_bass_kernel_spmd

    feat = np.asarray(feat, dtype=np.float32)
    ew = np.asarray(edge_weight, dtype=np.float32)
    beta = np.asarray(beta, dtype=np.float32)
    eps = np.asarray(eps, dtype=np.float32)

    zpad, core_idx, nb, net, K = _host_prep(src, dst, ew)
    tet = int(net.sum())

    key = (K, tuple(int(x) for x in net))
    if key not in _COMPILED:
        _COMPILED[key] = _build(net, K)
    nc = _COMPILED[key]

    featP = np.vstack([feat, np.zeros((1, D), np.float32)])  # pad row = 0
    beta2 = beta.reshape(1, 1)
    eps2 = eps.reshape(1, 1)

    in_maps = []
    for c in range(NCORES):
        src_pad, ewd, bp = core_idx[c]
        fe = featP[src_pad].reshape(tet, 128, D).transpose(1, 0, 2)
        zE = zpad[src_pad].reshape(tet, 128, K).transpose(1, 0, 2)
        import ml_dtypes

        in_maps.append(
            {
                "feat_edges": np.ascontiguousarray(fe).reshape(128, tet * D),
                "zpadE": np.ascontiguousarray(zE).reshape(128, tet * K),
                "ewp": ewd,
                "bitp": np.ascontiguousarray(bp).astype(ml_dtypes.bfloat16),
                "nbits": nb,
                "feat_my": np.ascontiguousarray(feat[c * SH : (c + 1) * SH]),
                "beta": beta2,
                "eps": eps2,
            }
        )

    res = run_bass_kernel_spmd(nc, in_maps, core_ids=list(range(NCORES)))
    out = np.concatenate([res.results[c]["out"] for c in range(NCORES)], axis=0)
    return out.astype(np.float32)



# revision 2
# speedup vs baseline: 1.6437x; 1.6437x over previous
"""AGNNConv distributed Bass kernel for 8 TRN2 NeuronCores (v6).

out = (1+eps)*feat + h,  h[d] = sum_{e: dst_e=d} p_e * norm_feat[src_e]
with p_e = edge-softmax grouped by src.

Algebra (softmax max-subtraction dropped -- identity in exact math):
    w_e = exp(beta*ew_e)
    z_n = sum_{e: src_e=n} w_e            # per NODE
    g_n = feat_n / (||feat_n|| * z_n)     # per NODE
    h_d = sum_{e: dst_e=d} w_e * g[src_e]
    out = (1+eps)*feat + h

v6 two-phase design (v5 was PE-latency-bound: 1354 serialized cold
matmuls at ~305ns because of the mm1->relu->mm2 chain, plus ~250us of
per-edge norm work on GpSimd/Vector):
  Phase 1 (node-sharded, ~tiny): each core computes g for its 6250
    nodes from feat_my + its nodes' own K-padded edge-weight rows.
    Per-edge norm/z work (11M elems/core) becomes per-node (0.8M).
  Host: gathers g[src_e] into the per-edge stream (pure indexing, same
    as the v5 feat gather) and builds the one-hot dst-scatter matrix.
  Phase 2 (dst-sharded edges): streams bf16 g-rows + fp8 one-hot st
    from HBM; msgq = g_e * w_e (one big DVE pass); h = st^T @ msgq as
    back-to-back PSUM-accumulating matmuls (no LDW stalls: fp8 FWL,
    no PSUM->SBUF round trip feeding the PE).
  DMA per core drops 63MB -> ~40MB, and no engine does per-edge
  norm work.
"""

import sys

sys.path.insert(0, "/opt/trn_rl_repo")

import numpy as np

N, E, D = 50000, 640000, 128
NCORES = 8
SH = N // NCORES            # 6250 dst nodes per core
HTILES = (SH + 127) // 128  # 49 dst tiles per core
SHP = HTILES * 128          # 6272 padded nodes per core

PAD_EW = -80.0              # exp(beta*PAD_EW) == 0 (inside ACT LUT range)


def _host_prep(src, dst, edge_weight):
    """Index/layout prep only (no float math on tensor values)."""
    src = np.asarray(src).astype(np.int64)
    dst = np.asarray(dst).astype(np.int64)
    ew = np.asarray(edge_weight).astype(np.float32)

    # ---- per-node src-grouped edge-weight rows (for z), fixed K ----
    deg = np.bincount(src, minlength=N)
    K = int(deg.max())
    order = np.argsort(src, kind="stable")
    starts = np.zeros(N + 1, np.int64)
    np.cumsum(deg, out=starts[1:])
    slot = np.arange(E, dtype=np.int64) - starts[src[order]]
    zpad = np.full((N, K), PAD_EW, np.float32)
    zpad[src[order], slot] = ew[order]

    # per-core phase-1 z rows: [128, HTILES*K], node-within-tile major
    zrows = []
    for c in range(NCORES):
        zm = np.full((SHP, K), PAD_EW, np.float32)
        zm[:SH] = zpad[c * SH : (c + 1) * SH]
        zrows.append(
            np.ascontiguousarray(
                zm.reshape(HTILES, 128, K).transpose(1, 0, 2)
            ).reshape(128, HTILES * K)
        )

    # ---- per-core edge grouping by dst tile ----
    owner = dst // SH
    dstl = dst - owner * SH
    dtile = dstl // 128
    dbit = dstl % 128

    counts = np.zeros((NCORES, HTILES), np.int64)
    np.add.at(counts, (owner, dtile), 1)
    net = (counts.max(axis=0) + 127) // 128  # [HTILES] edge tiles per dst tile
    net = np.maximum(net, 1)
    tet = int(net.sum())
    seg_off = np.zeros(HTILES, np.int64)
    seg_off[1:] = np.cumsum(net)[:-1]
    EPAD = tet * 128

    import ml_dtypes

    core_idx = []
    for c in range(NCORES):
        m = np.nonzero(owner == c)[0]
        key = dtile[m]
        korder = np.argsort(key, kind="stable")
        me = m[korder]
        keys = key[korder]
        kb = np.r_[0, np.nonzero(np.diff(keys))[0] + 1]
        sf = np.zeros(len(keys), np.int64)
        sf[kb] = kb
        np.maximum.accumulate(sf, out=sf)
        within = np.arange(len(keys)) - sf
        pos = seg_off[keys] * 128 + within

        src_pad = np.full(EPAD, N, np.int64)      # pad edges read zero row N
        ewp = np.full(EPAD, PAD_EW, np.float32)
        src_pad[pos] = src[me]
        ewp[pos] = ew[me]

        # one-hot dst-scatter matrix, fp8 (pad rows all-zero)
        stf = np.zeros((EPAD, 128), np.float32)
        stf[pos, dbit[me]] = 1.0
        stx = np.ascontiguousarray(
            stf.reshape(tet, 128, 128).transpose(1, 0, 2)
        ).reshape(128, tet * 128).astype(ml_dtypes.float8_e4m3)

        core_idx.append((src_pad, ewp.reshape(tet, 128).T.copy(), stx))

    return zrows, core_idx, net, K


_COMPILED = {}


def _build_phase1(K):
    import concourse.bass as bass
    import concourse.bacc as bacc
    from concourse import mybir, tile

    f32 = mybir.dt.float32
    bf16 = mybir.dt.bfloat16
    i32 = mybir.dt.int32
    AF = mybir.ActivationFunctionType
    ALU = mybir.AluOpType
    X = mybir.AxisListType.X

    NT = HTILES  # 49 node tiles

    nc = bacc.Bacc(None, debug=False)
    fm_ext = nc.dram_tensor("feat_my", [128, NT * D], f32, kind="ExternalInput")
    zr_ext = nc.dram_tensor("zrow", [128, NT * K], f32, kind="ExternalInput")
    beta_ext = nc.dram_tensor("beta", [1, 1], f32, kind="ExternalInput")
    g_ext = nc.dram_tensor("g", [128, NT * D], bf16, kind="ExternalOutput")

    with tile.TileContext(nc) as tc:
        with (
            tc.tile_pool(name="p1", bufs=1) as pp,
        ):
            beta_s = pp.tile([1, 1], f32, tag="beta_s")
            nc.sync.dma_start(out=beta_s[:], in_=beta_ext[:])
            beta_b = pp.tile([128, 1], f32, tag="beta_b")
            nc.gpsimd.partition_broadcast(beta_b[:], beta_s[:])

            fm = pp.tile([128, NT, D], f32, tag="fm")
            nc.sync.dma_start(
                out=fm[:].rearrange("p a b -> p (a b)"), in_=fm_ext[:]
            )
            zr = pp.tile([128, NT, K], f32, tag="zr")
            nc.sync.dma_start(
                out=zr[:].rearrange("p a b -> p (a b)"), in_=zr_ext[:]
            )

            # z = sum_k exp(beta * zrow_k)
            zx = pp.tile([128, NT, K], bf16, tag="zx")
            nc.scalar.activation(
                zx[:].rearrange("p a b -> p (a b)"),
                zr[:].rearrange("p a b -> p (a b)"),
                AF.Exp,
                scale=beta_b[:],
            )
            z = pp.tile([128, NT], f32, tag="z")
            nc.vector.tensor_reduce(z[:], zx[:], X, ALU.add)
            izn = pp.tile([128, NT], f32, tag="izn")
            nc.vector.reciprocal(izn[:], z[:])

            # ss = ||feat||^2 per node
            sq = pp.tile([128, NT, D], bf16, tag="sq")
            nc.gpsimd.tensor_tensor(
                sq[:].rearrange("p a b -> p (a b)"),
                fm[:].rearrange("p a b -> p (a b)"),
                fm[:].rearrange("p a b -> p (a b)"),
                ALU.mult,
            )
            ss = pp.tile([128, NT], f32, tag="ss")
            nc.vector.tensor_reduce(ss[:], sq[:], X, ALU.add)

            # rr = 1/sqrt(ss): int bit-hack + two Newton steps
            y0 = pp.tile([128, NT], f32, tag="y0")
            nc.vector.tensor_scalar(
                y0[:].bitcast(i32), ss[:].bitcast(i32),
                1, -1, op0=ALU.arith_shift_right, op1=ALU.bitwise_xor,
            )
            nc.vector.tensor_scalar(
                y0[:].bitcast(i32), y0[:].bitcast(i32),
                0x5F3759E0, None, op0=ALU.add,
            )
            u = pp.tile([128, NT], f32, tag="u")
            for _ in range(2):
                nc.vector.tensor_tensor(u[:], y0[:], y0[:], ALU.mult)
                nc.vector.tensor_tensor(u[:], u[:], ss[:], ALU.mult)
                nc.vector.tensor_scalar(
                    u[:], u[:], -0.5, 1.5, op0=ALU.mult, op1=ALU.add
                )
                nc.vector.tensor_tensor(y0[:], y0[:], u[:], ALU.mult)

            # rz = rr / z;  g = feat * rz
            rz = pp.tile([128, NT], f32, tag="rz")
            nc.vector.tensor_tensor(rz[:], y0[:], izn[:], ALU.mult)
            g = pp.tile([128, NT, D], bf16, tag="g")
            rzb = rz[:].unsqueeze(2).broadcast_to([128, NT, D])
            nc.vector.tensor_tensor(g[:], fm[:], rzb, ALU.mult)
            nc.sync.dma_start(
                out=g_ext[:], in_=g[:].rearrange("p a b -> p (a b)")
            )

    nc.finalize()
    return nc


def _build_phase2(net):
    import concourse.bass as bass
    import concourse.bacc as bacc
    from concourse import mybir, tile

    f32 = mybir.dt.float32
    bf16 = mybir.dt.bfloat16
    f8 = mybir.dt.float8e4
    AF = mybir.ActivationFunctionType
    ALU = mybir.AluOpType

    tet = int(net.sum())
    nmax = int(net.max())

    nc = bacc.Bacc(None, debug=False)
    ge_ext = nc.dram_tensor("ge", [128, tet * D], bf16, kind="ExternalInput")
    st_ext = nc.dram_tensor("stx", [128, tet * 128], f8, kind="ExternalInput")
    ewp_ext = nc.dram_tensor("ewp", [128, tet], f32, kind="ExternalInput")
    featmy_ext = nc.dram_tensor("feat_my", [SH, D], f32, kind="ExternalInput")
    beta_ext = nc.dram_tensor("beta", [1, 1], f32, kind="ExternalInput")
    eps_ext = nc.dram_tensor("eps", [1, 1], f32, kind="ExternalInput")
    out_ext = nc.dram_tensor("out", [SH, D], f32, kind="ExternalOutput")

    with tile.TileContext(nc) as tc:
        with (
            tc.tile_pool(name="persist", bufs=1) as pp,
            tc.tile_pool(name="msgp", bufs=3) as mpool,
            tc.tile_pool(name="stp", bufs=3) as stpool,
            tc.tile_pool(name="msgq", bufs=3) as mqpool,
            tc.tile_pool(name="outp", bufs=4) as opool,
            tc.tile_pool(name="hpsum", bufs=4, space="PSUM") as hpsum,
        ):
            beta_s = pp.tile([1, 1], f32, tag="beta_s")
            eps_s = pp.tile([1, 1], f32, tag="eps_s")
            nc.sync.dma_start(out=beta_s[:], in_=beta_ext[:])
            nc.sync.dma_start(out=eps_s[:], in_=eps_ext[:])
            beta_b = pp.tile([128, 1], f32, tag="beta_b")
            ep1_b = pp.tile([128, 1], f32, tag="ep1_b")
            nc.gpsimd.partition_broadcast(beta_b[:], beta_s[:])
            nc.gpsimd.partition_broadcast(ep1_b[:], eps_s[:])
            nc.vector.tensor_scalar_add(ep1_b[:], ep1_b[:], 1.0)

            # per-edge weight w = exp(beta*ew)
            wv = pp.tile([128, tet], f32, tag="wv")
            nc.sync.dma_start(out=wv[:], in_=ewp_ext[:])
            nc.scalar.activation(wv[:], wv[:], AF.Exp, scale=beta_b[:])

            offs = [0] * (HTILES + 1)
            for i in range(HTILES):
                offs[i + 1] = offs[i] + int(net[i])

            for i in range(HTILES):
                nh, T = int(net[i]), offs[i]
                rows = min(128, SH - i * 128)

                msg = mpool.tile([128, nmax, D], bf16, tag="msg")
                nc.sync.dma_start(
                    out=msg[:, :nh, :].rearrange("p a b -> p (a b)"),
                    in_=ge_ext[:, T * D : (T + nh) * D],
                )
                stt = stpool.tile([128, nmax, 128], f8, tag="stt")
                nc.sync.dma_start(
                    out=stt[:, :nh, :].rearrange("p a b -> p (a b)"),
                    in_=st_ext[:, T * 128 : (T + nh) * 128],
                )

                # msgq = g_e * w_e  (w broadcast along D)
                msgq = mqpool.tile([128, nmax, D], bf16, tag="msgq")
                wb = wv[:, T : T + nh].unsqueeze(2).broadcast_to([128, nh, D])
                nc.vector.tensor_tensor(
                    msgq[:, :nh, :], msg[:, :nh, :], wb, ALU.mult
                )

                # h tile: back-to-back PSUM-accumulating matmuls
                hp = hpsum.tile([128, D], f32, tag="hp")
                for t in range(nh):
                    nc.tensor.matmul(
                        hp[:],
                        stt[:, t, :],
                        msgq[:, t, :],
                        start=(t == 0),
                        stop=(t == nh - 1),
                    )

                # out = hp + (1+eps)*feat_my
                ftm = opool.tile([128, D], f32, tag="ftm")
                nc.sync.dma_start(
                    out=ftm[:rows, :],
                    in_=featmy_ext[i * 128 : i * 128 + rows, :],
                )
                fts = opool.tile([128, D], f32, tag="fts")
                nc.scalar.activation(
                    fts[:rows, :], ftm[:rows, :], AF.Copy,
                    scale=ep1_b[:rows, :],
                )
                ot = opool.tile([128, D], f32, tag="ot")
                nc.vector.tensor_tensor(
                    ot[:rows, :], fts[:rows, :], hp[:rows, :], ALU.add
                )
                nc.sync.dma_start(
                    out=out_ext[i * 128 : i * 128 + rows, :], in_=ot[:rows, :]
                )

    nc.finalize()
    return nc


def kernel(feat, edge_weight, src, dst, beta, eps):
    from concourse.bass_utils import run_bass_kernel_spmd
    import ml_dtypes

    feat = np.asarray(feat, dtype=np.float32)
    ew = np.asarray(edge_weight, dtype=np.float32)
    beta = np.asarray(beta, dtype=np.float32)
    eps = np.asarray(eps, dtype=np.float32)

    zrows, core_idx, net, K = _host_prep(src, dst, ew)
    tet = int(net.sum())

    key = (K, tuple(int(x) for x in net))
    if key not in _COMPILED:
        _COMPILED[key] = (_build_phase1(K), _build_phase2(net))
    nc1, nc2 = _COMPILED[key]

    beta2 = beta.reshape(1, 1)
    eps2 = eps.reshape(1, 1)

    # ---------------- phase 1: per-node g ----------------
    in1 = []
    for c in range(NCORES):
        fmp = np.zeros((SHP, D), np.float32)
        fmp[:SH] = feat[c * SH : (c + 1) * SH]
        fmt = np.ascontiguousarray(
            fmp.reshape(HTILES, 128, D).transpose(1, 0, 2)
        ).reshape(128, HTILES * D)
        in1.append({"feat_my": fmt, "zrow": zrows[c], "beta": beta2})

    res1 = run_bass_kernel_spmd(nc1, in1, core_ids=list(range(NCORES)))
    gfull = np.empty((N + 1, D), dtype=ml_dtypes.bfloat16)
    for c in range(NCORES):
        gc = np.asarray(res1.results[c]["g"]).reshape(128, HTILES, D)
        gfull[c * SH : (c + 1) * SH] = gc.transpose(1, 0, 2).reshape(SHP, D)[:SH]
    gfull[N] = 0  # pad row

    # ---------------- host gather of g[src_e] ----------------
    in2 = []
    for c in range(NCORES):
        src_pad, ewd, stx = core_idx[c]
        ge = np.ascontiguousarray(
            gfull[src_pad].reshape(tet, 128, D).transpose(1, 0, 2)
        ).reshape(128, tet * D)
        in2.append(
            {
                "ge": ge,
                "stx": stx,
                "ewp": ewd,
                "feat_my": np.ascontiguousarray(feat[c * SH : (c + 1) * SH]),
                "beta": beta2,
                "eps": eps2,
            }
        )

    res2 = run_bass_kernel_spmd(nc2, in2, core_ids=list(range(NCORES)))
    out = np.concatenate([res2.results[c]["out"] for c in range(NCORES)], axis=0)
    return out.astype(np.float32)


# revision 4
# speedup vs baseline: 2.3621x; 1.4371x over previous
"""AGNNConv distributed Bass kernel for 8 TRN2 NeuronCores (v7).

out = (1+eps)*feat + h,  h[d] = sum_{e: dst_e=d} p_e * norm_feat[src_e]
with p_e = edge-softmax grouped by src.

Algebra:
    w_e = exp(beta*ew_e)
    z_n = sum_{e: src_e=n} w_e            # per NODE
    g_n = feat_n / (||feat_n|| * z_n)     # per NODE
    h_d = sum_{e: dst_e=d} w_e * g[src_e]
    out = (1+eps)*feat + h

v7 (v6 was PE-cold + DVE-broadcast-bound + sub-rate DMA):
  Phase 1 (node-sharded): g64 = 64*g and o0 = (1+eps)*feat per node.
  Host: gathers g64[src_e] (fp8) per edge; scatters RAW ew_e values
    (pure relayout) into one-hot slots of an fp8 matrix straw with
    -80 fill.
  Phase 2: ONE batched ScalarE exp turns straw into the w-scaled
    scatter matrix in place: stw[e,d] = exp(beta*straw - ln64)
    = w_e/64 at one-hot slots, ~0 elsewhere. h-tile = stw^T @ ge64
    as back-to-back PSUM-accumulating matmuls. No per-edge DVE work
    at all; DVE only does the 49 output adds. All big streams fp8,
    DMA batched 4 dst-tiles per transfer.
"""

import sys

sys.path.insert(0, "/opt/trn_rl_repo")

import numpy as np

N, E, D = 50000, 640000, 128
NCORES = 8
SH = N // NCORES            # 6250 dst nodes per core
HTILES = (SH + 127) // 128  # 49 dst tiles per core
SHP = HTILES * 128          # 6272 padded nodes per core

PAD_EW = -80.0              # exp(beta*PAD_EW) == 0 (inside ACT LUT range)
LN64 = 4.1588830833596715
GRP = 4                     # dst tiles per DMA/ACT batch


def _host_prep(src, dst, edge_weight):
    """Index/layout prep only (no float math on tensor values)."""
    import ml_dtypes

    src = np.asarray(src).astype(np.int64)
    dst = np.asarray(dst).astype(np.int64)
    ew = np.asarray(edge_weight).astype(np.float32)

    # ---- per-node src-grouped edge-weight rows (for z), fixed K ----
    deg = np.bincount(src, minlength=N)
    K = int(deg.max())
    order = np.argsort(src, kind="stable")
    starts = np.zeros(N + 1, np.int64)
    np.cumsum(deg, out=starts[1:])
    slot = np.arange(E, dtype=np.int64) - starts[src[order]]
    zpad = np.full((N, K), PAD_EW, np.float32)
    zpad[src[order], slot] = ew[order]

    # per-core phase-1 z rows: [128, HTILES*K] bf16
    zrows = []
    for c in range(NCORES):
        zm = np.full((SHP, K), PAD_EW, np.float32)
        zm[:SH] = zpad[c * SH : (c + 1) * SH]
        zrows.append(
            np.ascontiguousarray(
                zm.reshape(HTILES, 128, K).transpose(1, 0, 2)
            ).reshape(128, HTILES * K).astype(ml_dtypes.bfloat16)
        )

    # ---- per-core edge grouping by dst tile ----
    owner = dst // SH
    dstl = dst - owner * SH
    dtile = dstl // 128
    dbit = dstl % 128

    counts = np.zeros((NCORES, HTILES), np.int64)
    np.add.at(counts, (owner, dtile), 1)
    net = (counts.max(axis=0) + 127) // 128  # [HTILES] edge tiles per dst tile
    net = np.maximum(net, 1)
    tet = int(net.sum())
    seg_off = np.zeros(HTILES, np.int64)
    seg_off[1:] = np.cumsum(net)[:-1]
    EPAD = tet * 128

    core_idx = []
    for c in range(NCORES):
        m = np.nonzero(owner == c)[0]
        key = dtile[m]
        korder = np.argsort(key, kind="stable")
        me = m[korder]
        keys = key[korder]
        kb = np.r_[0, np.nonzero(np.diff(keys))[0] + 1]
        sf = np.zeros(len(keys), np.int64)
        sf[kb] = kb
        np.maximum.accumulate(sf, out=sf)
        within = np.arange(len(keys)) - sf
        pos = seg_off[keys] * 128 + within

        src_pad = np.full(EPAD, N, np.int64)      # pad edges read zero row N

        # raw-ew scatter matrix: ew_e at [e, dbit_e], -80 fill (fp8)
        stf = np.full((EPAD, 128), PAD_EW, np.float32)
        stf[pos, dbit[me]] = ew[me]
        straw = np.ascontiguousarray(
            stf.reshape(tet, 128, 128).transpose(1, 0, 2)
        ).reshape(128, tet * 128).astype(ml_dtypes.float8_e4m3)

        src_pad[pos] = src[me]
        core_idx.append((src_pad, straw))

    return zrows, core_idx, net, K


_COMPILED = {}


def _build_phase1(K):
    import concourse.bass as bass
    import concourse.bacc as bacc
    from concourse import mybir, tile

    f32 = mybir.dt.float32
    bf16 = mybir.dt.bfloat16
    i32 = mybir.dt.int32
    AF = mybir.ActivationFunctionType
    ALU = mybir.AluOpType
    X = mybir.AxisListType.X

    NT = HTILES       # 49 node tiles
    CH = 13           # node tiles per pipeline chunk
    NCH = (NT + CH - 1) // CH

    nc = bacc.Bacc(None, debug=False)
    fm_ext = nc.dram_tensor("feat_my", [128, NT * D], bf16, kind="ExternalInput")
    zr_ext = nc.dram_tensor("zrow", [128, NT * K], bf16, kind="ExternalInput")
    beta_ext = nc.dram_tensor("beta", [1, 1], f32, kind="ExternalInput")
    eps_ext = nc.dram_tensor("eps", [1, 1], f32, kind="ExternalInput")
    g_ext = nc.dram_tensor("g64", [128, NT * D], bf16, kind="ExternalOutput")
    o0_ext = nc.dram_tensor("o0", [128, NT * D], bf16, kind="ExternalOutput")

    with tile.TileContext(nc) as tc:
        with (
            tc.tile_pool(name="pp", bufs=1) as pp,
            tc.tile_pool(name="fmp", bufs=2) as fmp,
            tc.tile_pool(name="zrp", bufs=2) as zrp,
            tc.tile_pool(name="sqp", bufs=2) as sqp,
            tc.tile_pool(name="smp", bufs=2 * 8) as smp,
            tc.tile_pool(name="gp", bufs=2) as gp,
            tc.tile_pool(name="op", bufs=2) as op,
        ):
            beta_s = pp.tile([1, 1], f32, tag="beta_s")
            eps_s = pp.tile([1, 1], f32, tag="eps_s")
            nc.sync.dma_start(out=beta_s[:], in_=beta_ext[:])
            nc.sync.dma_start(out=eps_s[:], in_=eps_ext[:])
            beta_b = pp.tile([128, 1], f32, tag="beta_b")
            ep1_b = pp.tile([128, 1], f32, tag="ep1_b")
            nc.gpsimd.partition_broadcast(beta_b[:], beta_s[:])
            nc.gpsimd.partition_broadcast(ep1_b[:], eps_s[:])
            nc.vector.tensor_scalar_add(ep1_b[:], ep1_b[:], 1.0)

            for ci in range(NCH):
                t0 = ci * CH
                nt = min(CH, NT - t0)
                fm = fmp.tile([128, CH, D], bf16, tag="fm")
                nc.sync.dma_start(
                    out=fm[:, :nt, :].rearrange("p a b -> p (a b)"),
                    in_=fm_ext[:, t0 * D : (t0 + nt) * D],
                )
                zr = zrp.tile([128, CH, K], bf16, tag="zr")
                nc.sync.dma_start(
                    out=zr[:, :nt, :].rearrange("p a b -> p (a b)"),
                    in_=zr_ext[:, t0 * K : (t0 + nt) * K],
                )

                # z = sum_k exp(beta * zrow_k);  izn = 64/z
                zx = zrp.tile([128, CH, K], bf16, tag="zx")
                nc.scalar.activation(
                    zx[:, :nt, :].rearrange("p a b -> p (a b)"),
                    zr[:, :nt, :].rearrange("p a b -> p (a b)"),
                    AF.Exp,
                    scale=beta_b[:],
                )
                z = smp.tile([128, CH], f32, tag="z")
                nc.vector.tensor_reduce(z[:, :nt], zx[:, :nt, :], X, ALU.add)
                izn = smp.tile([128, CH], f32, tag="izn")
                nc.vector.reciprocal(izn[:, :nt], z[:, :nt])
                nc.vector.tensor_scalar(
                    izn[:, :nt], izn[:, :nt], 64.0, None, op0=ALU.mult
                )

                # ss = ||feat||^2 per node (bf16 squares -> 2x mode)
                sq = sqp.tile([128, CH, D], bf16, tag="sq")
                nc.vector.tensor_tensor(
                    sq[:, :nt, :].rearrange("p a b -> p (a b)"),
                    fm[:, :nt, :].rearrange("p a b -> p (a b)"),
                    fm[:, :nt, :].rearrange("p a b -> p (a b)"),
                    ALU.mult,
                )
                ss = smp.tile([128, CH], f32, tag="ss")
                nc.vector.tensor_reduce(ss[:, :nt], sq[:, :nt, :], X, ALU.add)

                # rr = 1/sqrt(ss): bit-hack + two Newton steps
                y0 = smp.tile([128, CH], f32, tag="y0")
                nc.vector.tensor_scalar(
                    y0[:, :nt].bitcast(i32), ss[:, :nt].bitcast(i32),
                    1, -1, op0=ALU.arith_shift_right, op1=ALU.bitwise_xor,
                )
                nc.vector.tensor_scalar(
                    y0[:, :nt].bitcast(i32), y0[:, :nt].bitcast(i32),
                    0x5F3759E0, None, op0=ALU.add,
                )
                u = smp.tile([128, CH], f32, tag="u")
                for _ in range(2):
                    nc.vector.tensor_tensor(
                        u[:, :nt], y0[:, :nt], y0[:, :nt], ALU.mult
                    )
                    nc.vector.tensor_tensor(
                        u[:, :nt], u[:, :nt], ss[:, :nt], ALU.mult
                    )
                    nc.vector.tensor_scalar(
                        u[:, :nt], u[:, :nt], -0.5, 1.5, op0=ALU.mult, op1=ALU.add
                    )
                    nc.vector.tensor_tensor(
                        y0[:, :nt], y0[:, :nt], u[:, :nt], ALU.mult
                    )

                # rz = 64*rr/z;  g64 = feat*rz;  o0 = (1+eps)*feat
                rz = smp.tile([128, CH], f32, tag="rz")
                nc.vector.tensor_tensor(rz[:, :nt], y0[:, :nt], izn[:, :nt], ALU.mult)
                g = gp.tile([128, CH, D], bf16, tag="g")
                rzb = rz[:, :nt].unsqueeze(2).broadcast_to([128, nt, D])
                nc.vector.tensor_tensor(g[:, :nt, :], fm[:, :nt, :], rzb, ALU.mult)
                nc.sync.dma_start(
                    out=g_ext[:, t0 * D : (t0 + nt) * D],
                    in_=g[:, :nt, :].rearrange("p a b -> p (a b)"),
                )
                o0 = op.tile([128, CH, D], bf16, tag="o0")
                epb = ep1_b[:].unsqueeze(2).broadcast_to([128, nt, D])
                nc.vector.tensor_tensor(o0[:, :nt, :], fm[:, :nt, :], epb, ALU.mult)
                nc.sync.dma_start(
                    out=o0_ext[:, t0 * D : (t0 + nt) * D],
                    in_=o0[:, :nt, :].rearrange("p a b -> p (a b)"),
                )

    nc.finalize()
    return nc


def _build_phase2(net):
    import concourse.bass as bass
    import concourse.bacc as bacc
    from concourse import mybir, tile

    f32 = mybir.dt.float32
    bf16 = mybir.dt.bfloat16
    f8 = mybir.dt.float8e4
    AF = mybir.ActivationFunctionType
    ALU = mybir.AluOpType

    tet = int(net.sum())
    nmax = int(net.max())

    nc = bacc.Bacc(None, debug=False)
    ge_ext = nc.dram_tensor("ge", [128, tet * D], f8, kind="ExternalInput")
    st_ext = nc.dram_tensor("straw", [128, tet * 128], f8, kind="ExternalInput")
    o0_ext = nc.dram_tensor("o0", [128, HTILES * D], bf16, kind="ExternalInput")
    beta_ext = nc.dram_tensor("beta", [1, 1], f32, kind="ExternalInput")
    out_ext = nc.dram_tensor("out", [128, HTILES * D], bf16, kind="ExternalOutput")

    with tile.TileContext(nc) as tc:
        with (
            tc.tile_pool(name="persist", bufs=1) as pp,
            tc.tile_pool(name="gep", bufs=3) as gepool,
            tc.tile_pool(name="stp", bufs=3) as stpool,
            tc.tile_pool(name="stw", bufs=3) as stwpool,
            tc.tile_pool(name="outp", bufs=8) as opool,
            tc.tile_pool(name="hpsum", bufs=8, space="PSUM") as hpsum,
        ):
            beta_s = pp.tile([1, 1], f32, tag="beta_s")
            nc.sync.dma_start(out=beta_s[:], in_=beta_ext[:])
            beta_b = pp.tile([128, 1], f32, tag="beta_b")
            nc.gpsimd.partition_broadcast(beta_b[:], beta_s[:])
            bl64 = pp.tile([128, 1], f32, tag="bl64")
            nc.vector.memset(bl64[:], -LN64)

            # (1+eps)*feat rows, entire shard resident
            o0 = pp.tile([128, HTILES * D], bf16, tag="o0")
            nc.sync.dma_start(out=o0[:], in_=o0_ext[:])

            offs = [0] * (HTILES + 1)
            for i in range(HTILES):
                offs[i + 1] = offs[i] + int(net[i])

            for i0 in range(0, HTILES, GRP):
                tiles = list(range(i0, min(i0 + GRP, HTILES)))
                gnh = sum(int(net[i]) for i in tiles)
                T0 = offs[i0]

                ge = gepool.tile([128, GRP * nmax, D], f8, tag="ge")
                nc.sync.dma_start(
                    out=ge[:, :gnh, :].rearrange("p a b -> p (a b)"),
                    in_=ge_ext[:, T0 * D : (T0 + gnh) * D],
                )
                straw = stpool.tile([128, GRP * nmax, 128], f8, tag="straw")
                nc.sync.dma_start(
                    out=straw[:, :gnh, :].rearrange("p a b -> p (a b)"),
                    in_=st_ext[:, T0 * 128 : (T0 + gnh) * 128],
                )

                # stw = exp(beta*straw - ln64): w_e/64 one-hot-placed
                stw = stwpool.tile([128, GRP * nmax, 128], bf16, tag="stw")
                nc.scalar.activation(
                    stw[:, :gnh, :].rearrange("p a b -> p (a b)"),
                    straw[:, :gnh, :].rearrange("p a b -> p (a b)"),
                    AF.Exp,
                    bias=bl64[:],
                    scale=beta_b[:],
                )

                for i in tiles:
                    nh, T = int(net[i]), offs[i]
                    hp = hpsum.tile([128, D], f32, tag="hp")
                    for t in range(nh):
                        tl = T - T0 + t
                        nc.tensor.matmul(
                            hp[:],
                            stw[:, tl, :],
                            ge[:, tl, :],
                            start=(t == 0),
                            stop=(t == nh - 1),
                        )
                    ot = opool.tile([128, D], bf16, tag="ot")
                    nc.vector.tensor_tensor(
                        ot[:], o0[:, i * D : (i + 1) * D], hp[:], ALU.add
                    )
                    nc.sync.dma_start(
                        out=out_ext[:, i * D : (i + 1) * D], in_=ot[:]
                    )

    nc.finalize()
    return nc


def kernel(feat, edge_weight, src, dst, beta, eps):
    from concourse.bass_utils import run_bass_kernel_spmd
    import ml_dtypes

    feat = np.asarray(feat, dtype=np.float32)
    ew = np.asarray(edge_weight, dtype=np.float32)
    beta = np.asarray(beta, dtype=np.float32)
    eps = np.asarray(eps, dtype=np.float32)

    zrows, core_idx, net, K = _host_prep(src, dst, ew)
    tet = int(net.sum())

    key = (K, tuple(int(x) for x in net))
    if key not in _COMPILED:
        _COMPILED[key] = (_build_phase1(K), _build_phase2(net))
    nc1, nc2 = _COMPILED[key]

    beta2 = beta.reshape(1, 1)
    eps2 = eps.reshape(1, 1)

    # ---------------- phase 1: per-node g64, o0 ----------------
    in1 = []
    for c in range(NCORES):
        fmp = np.zeros((SHP, D), np.float32)
        fmp[:SH] = feat[c * SH : (c + 1) * SH]
        fmt = np.ascontiguousarray(
            fmp.reshape(HTILES, 128, D).transpose(1, 0, 2)
        ).reshape(128, HTILES * D).astype(ml_dtypes.bfloat16)
        in1.append(
            {"feat_my": fmt, "zrow": zrows[c], "beta": beta2, "eps": eps2}
        )

    res1 = run_bass_kernel_spmd(nc1, in1, core_ids=list(range(NCORES)))
    gfull = np.empty((N + 1, D), dtype=ml_dtypes.float8_e4m3)
    o0s = []
    for c in range(NCORES):
        gc = np.asarray(res1.results[c]["g64"]).reshape(128, HTILES, D)
        gfull[c * SH : (c + 1) * SH] = (
            gc.transpose(1, 0, 2).reshape(SHP, D)[:SH].astype(ml_dtypes.float8_e4m3)
        )
        o0s.append(np.asarray(res1.results[c]["o0"]))
    gfull[N] = 0  # pad row

    # ---------------- host gather of g64[src_e] ----------------
    in2 = []
    for c in range(NCORES):
        src_pad, straw = core_idx[c]
        ge = np.ascontiguousarray(
            gfull[src_pad].reshape(tet, 128, D).transpose(1, 0, 2)
        ).reshape(128, tet * D)
        in2.append({"ge": ge, "straw": straw, "o0": o0s[c], "beta": beta2})

    res2 = run_bass_kernel_spmd(nc2, in2, core_ids=list(range(NCORES)))
    out = np.empty((N, D), np.float32)
    for c in range(NCORES):
        oc = np.asarray(res2.results[c]["out"]).reshape(128, HTILES, D)
        out[c * SH : (c + 1) * SH] = (
            oc.transpose(1, 0, 2).reshape(SHP, D)[:SH].astype(np.float32)
        )
    return out


# revision 6
# speedup vs baseline: 2.8953x; 1.2257x over previous
"""AGNNConv distributed Bass kernel for 8 TRN2 NeuronCores (v8).

out = (1+eps)*feat + h,  h[d] = sum_{e: dst_e=d} p_e * norm_feat[src_e]
with p_e = edge-softmax grouped by src.

Algebra:
    w_e = exp(beta*ew_e)
    z_n = sum_{e: src_e=n} w_e            # per NODE
    g_n = feat_n / (||feat_n|| * z_n)     # per NODE
    h_d = sum_{e: dst_e=d} w_e * g[src_e]
    out = (1+eps)*feat + h

v8 (v7 lost ~40us to group-boundary bubbles: ACT exp waited on the
strided straw DMA, PE waited on ACT, out-DMAs were 32KB):
  Phase 1 (node-sharded): g64 = 64*g (bf16 -> host casts fp8) and
    o0 = (1+eps)*feat per node. o0 on ScalarE; beta/eps replicated
    to [128,1] on host so no GpSimd broadcasts.
  Host: gathers g64[src_e] per edge; scatters RAW ew_e values (pure
    relayout) into one-hot slots of fp8 straw with -80 fill. Both
    per-edge streams stored GROUP-CONTIGUOUS in DRAM (4 dst tiles
    padded to GMAX edge tiles) so each group is one linear read.
  Phase 2: per group: straw DMA first, then ge; one batched ScalarE
    exp builds stw[e,d] = exp(beta*straw - ln64) = w_e/64 one-hot
    placed; h-tile = stw^T @ ge64 as PSUM-accumulating matmuls;
    ot = o0 + hp per tile into a group buffer, one out DMA per group.
"""

import sys

sys.path.insert(0, "/opt/trn_rl_repo")

import numpy as np

N, E, D = 50000, 640000, 128
NCORES = 8
SH = N // NCORES            # 6250 dst nodes per core
HTILES = (SH + 127) // 128  # 49 dst tiles per core
SHP = HTILES * 128          # 6272 padded nodes per core

PAD_EW = -80.0              # exp(beta*PAD_EW) == 0 (inside ACT LUT range)
LN64 = 4.1588830833596715
GRP = 4                     # dst tiles per DMA/ACT batch
NG = (HTILES + GRP - 1) // GRP  # 13 groups


def _host_prep(src, dst, edge_weight):
    """Index/layout prep only (no float math on tensor values)."""
    import ml_dtypes

    src = np.asarray(src).astype(np.int64)
    dst = np.asarray(dst).astype(np.int64)
    ew = np.asarray(edge_weight).astype(np.float32)

    # ---- per-node src-grouped edge-weight rows (for z), fixed K ----
    deg = np.bincount(src, minlength=N)
    K = int(deg.max())
    order = np.argsort(src, kind="stable")
    starts = np.zeros(N + 1, np.int64)
    np.cumsum(deg, out=starts[1:])
    slot = np.arange(E, dtype=np.int64) - starts[src[order]]
    zpad = np.full((N, K), PAD_EW, np.float32)
    zpad[src[order], slot] = ew[order]

    # per-core phase-1 z rows: [128, HTILES*K] bf16
    zrows = []
    for c in range(NCORES):
        zm = np.full((SHP, K), PAD_EW, np.float32)
        zm[:SH] = zpad[c * SH : (c + 1) * SH]
        zrows.append(
            np.ascontiguousarray(
                zm.reshape(HTILES, 128, K).transpose(1, 0, 2)
            ).reshape(128, HTILES * K).astype(ml_dtypes.bfloat16)
        )

    # ---- per-core edge grouping by dst tile ----
    owner = dst // SH
    dstl = dst - owner * SH
    dtile = dstl // 128
    dbit = dstl % 128

    counts = np.zeros((NCORES, HTILES), np.int64)
    np.add.at(counts, (owner, dtile), 1)
    net = (counts.max(axis=0) + 127) // 128  # [HTILES] edge tiles per dst tile
    net = np.maximum(net, 1)

    # group-local layout: tiles of group g at local cumsum offsets,
    # each group padded to GMAX edge tiles
    loff = np.zeros(HTILES, np.int64)   # edge-tile offset of dst tile
    gsum = np.zeros(NG, np.int64)       # edge tiles per group
    for g in range(NG):
        o = 0
        for i in range(g * GRP, min((g + 1) * GRP, HTILES)):
            loff[i] = o
            o += int(net[i])
        gsum[g] = o
    GMAX = int(gsum.max())
    EPAD = NG * GMAX * 128

    core_idx = []
    for c in range(NCORES):
        m = np.nonzero(owner == c)[0]
        key = dtile[m]
        korder = np.argsort(key, kind="stable")
        me = m[korder]
        keys = key[korder]
        kb = np.r_[0, np.nonzero(np.diff(keys))[0] + 1]
        sf = np.zeros(len(keys), np.int64)
        sf[kb] = kb
        np.maximum.accumulate(sf, out=sf)
        within = np.arange(len(keys)) - sf
        grp = keys // GRP
        pos = (grp * GMAX + loff[keys]) * 128 + within

        src_pad = np.full(EPAD, N, np.int64)      # pad edges read zero row N

        # raw-ew scatter matrix: ew_e at [e, dbit_e], -80 fill (fp8)
        # layout [NG*128, GMAX*128]: group blocks fully contiguous
        stf = np.full((EPAD, 128), PAD_EW, np.float32)
        stf[pos, dbit[me]] = ew[me]
        straw = np.ascontiguousarray(
            stf.reshape(NG, GMAX, 128, 128).transpose(0, 2, 1, 3)
        ).reshape(NG * 128, GMAX * 128).astype(ml_dtypes.float8_e4m3)
        # zero the pure-pad edge tiles' straw? they stay -80 -> exp ~0, but
        # MMs never touch slots >= gsum[g]; leave as is.

        src_pad[pos] = src[me]
        core_idx.append((src_pad, straw))

    return zrows, core_idx, net, K, loff, gsum, GMAX


_COMPILED = {}


def _build_phase1(K):
    import concourse.bass as bass
    import concourse.bacc as bacc
    from concourse import mybir, tile

    f32 = mybir.dt.float32
    bf16 = mybir.dt.bfloat16
    i32 = mybir.dt.int32
    AF = mybir.ActivationFunctionType
    ALU = mybir.AluOpType
    X = mybir.AxisListType.X

    NT = HTILES       # 49 node tiles
    CH = 13           # node tiles per pipeline chunk
    NCH = (NT + CH - 1) // CH

    nc = bacc.Bacc(None, debug=False)
    fm_ext = nc.dram_tensor("feat_my", [128, NT * D], bf16, kind="ExternalInput")
    zr_ext = nc.dram_tensor("zrow", [128, NT * K], bf16, kind="ExternalInput")
    beta_ext = nc.dram_tensor("beta128", [128, 1], f32, kind="ExternalInput")
    eps_ext = nc.dram_tensor("eps128", [128, 1], f32, kind="ExternalInput")
    g_ext = nc.dram_tensor("g64", [128, NT * D], bf16, kind="ExternalOutput")
    o0_ext = nc.dram_tensor("o0", [128, NT * D], bf16, kind="ExternalOutput")

    with tile.TileContext(nc) as tc:
        with (
            tc.tile_pool(name="pp", bufs=1) as pp,
            tc.tile_pool(name="fmp", bufs=2) as fmp,
            tc.tile_pool(name="zrp", bufs=2) as zrp,
            tc.tile_pool(name="sqp", bufs=2) as sqp,
            tc.tile_pool(name="smp", bufs=2 * 8) as smp,
            tc.tile_pool(name="gp", bufs=2) as gp,
            tc.tile_pool(name="op", bufs=2) as op,
        ):
            beta_b = pp.tile([128, 1], f32, tag="beta_b")
            nc.sync.dma_start(out=beta_b[:], in_=beta_ext[:])
            ep1_b = pp.tile([128, 1], f32, tag="ep1_b")
            nc.sync.dma_start(out=ep1_b[:], in_=eps_ext[:])
            nc.vector.tensor_scalar_add(ep1_b[:], ep1_b[:], 1.0)

            for ci in range(NCH):
                t0 = ci * CH
                nt = min(CH, NT - t0)
                fm = fmp.tile([128, CH, D], bf16, tag="fm")
                nc.sync.dma_start(
                    out=fm[:, :nt, :].rearrange("p a b -> p (a b)"),
                    in_=fm_ext[:, t0 * D : (t0 + nt) * D],
                )
                zr = zrp.tile([128, CH, K], bf16, tag="zr")
                nc.sync.dma_start(
                    out=zr[:, :nt, :].rearrange("p a b -> p (a b)"),
                    in_=zr_ext[:, t0 * K : (t0 + nt) * K],
                )

                # z = sum_k exp(beta * zrow_k);  izn = 64/z
                zx = zrp.tile([128, CH, K], bf16, tag="zx")
                nc.scalar.activation(
                    zx[:, :nt, :].rearrange("p a b -> p (a b)"),
                    zr[:, :nt, :].rearrange("p a b -> p (a b)"),
                    AF.Exp,
                    scale=beta_b[:],
                )
                z = smp.tile([128, CH], f32, tag="z")
                nc.vector.tensor_reduce(z[:, :nt], zx[:, :nt, :], X, ALU.add)
                izn = smp.tile([128, CH], f32, tag="izn")
                nc.vector.reciprocal(izn[:, :nt], z[:, :nt])
                nc.vector.tensor_scalar(
                    izn[:, :nt], izn[:, :nt], 64.0, None, op0=ALU.mult
                )

                # ss = ||feat||^2 per node (bf16 squares -> 2x mode)
                sq = sqp.tile([128, CH, D], bf16, tag="sq")
                nc.vector.tensor_tensor(
                    sq[:, :nt, :].rearrange("p a b -> p (a b)"),
                    fm[:, :nt, :].rearrange("p a b -> p (a b)"),
                    fm[:, :nt, :].rearrange("p a b -> p (a b)"),
                    ALU.mult,
                )
                ss = smp.tile([128, CH], f32, tag="ss")
                nc.vector.tensor_reduce(ss[:, :nt], sq[:, :nt, :], X, ALU.add)

                # rr = 1/sqrt(ss): bit-hack + two Newton steps
                y0 = smp.tile([128, CH], f32, tag="y0")
                nc.vector.tensor_scalar(
                    y0[:, :nt].bitcast(i32), ss[:, :nt].bitcast(i32),
                    1, -1, op0=ALU.arith_shift_right, op1=ALU.bitwise_xor,
                )
                nc.vector.tensor_scalar(
                    y0[:, :nt].bitcast(i32), y0[:, :nt].bitcast(i32),
                    0x5F3759E0, None, op0=ALU.add,
                )
                u = smp.tile([128, CH], f32, tag="u")
                for _ in range(2):
                    nc.vector.tensor_tensor(
                        u[:, :nt], y0[:, :nt], y0[:, :nt], ALU.mult
                    )
                    nc.vector.tensor_tensor(
                        u[:, :nt], u[:, :nt], ss[:, :nt], ALU.mult
                    )
                    nc.vector.tensor_scalar(
                        u[:, :nt], u[:, :nt], -0.5, 1.5, op0=ALU.mult, op1=ALU.add
                    )
                    nc.vector.tensor_tensor(
                        y0[:, :nt], y0[:, :nt], u[:, :nt], ALU.mult
                    )

                # rz = 64*rr/z;  g64 = feat*rz (DVE);  o0 = (1+eps)*feat (ACT)
                rz = smp.tile([128, CH], f32, tag="rz")
                nc.vector.tensor_tensor(rz[:, :nt], y0[:, :nt], izn[:, :nt], ALU.mult)
                g = gp.tile([128, CH, D], bf16, tag="g")
                rzb = rz[:, :nt].unsqueeze(2).broadcast_to([128, nt, D])
                nc.vector.tensor_tensor(g[:, :nt, :], fm[:, :nt, :], rzb, ALU.mult)
                nc.sync.dma_start(
                    out=g_ext[:, t0 * D : (t0 + nt) * D],
                    in_=g[:, :nt, :].rearrange("p a b -> p (a b)"),
                )
                o0 = op.tile([128, CH, D], bf16, tag="o0")
                nc.scalar.activation(
                    o0[:, :nt, :].rearrange("p a b -> p (a b)"),
                    fm[:, :nt, :].rearrange("p a b -> p (a b)"),
                    AF.Copy,
                    scale=ep1_b[:],
                )
                nc.sync.dma_start(
                    out=o0_ext[:, t0 * D : (t0 + nt) * D],
                    in_=o0[:, :nt, :].rearrange("p a b -> p (a b)"),
                )

    nc.finalize()
    return nc


def _build_phase2(net, loff, gsum, GMAX):
    import concourse.bass as bass
    import concourse.bacc as bacc
    from concourse import mybir, tile

    f32 = mybir.dt.float32
    bf16 = mybir.dt.bfloat16
    f8 = mybir.dt.float8e4
    AF = mybir.ActivationFunctionType
    ALU = mybir.AluOpType

    nc = bacc.Bacc(None, debug=False)
    ge_ext = nc.dram_tensor("ge", [NG * 128, GMAX * D], f8, kind="ExternalInput")
    st_ext = nc.dram_tensor("straw", [NG * 128, GMAX * 128], f8, kind="ExternalInput")
    o0_ext = nc.dram_tensor("o0", [128, HTILES * D], bf16, kind="ExternalInput")
    beta_ext = nc.dram_tensor("beta128", [128, 1], f32, kind="ExternalInput")
    out_ext = nc.dram_tensor("out", [128, HTILES * D], bf16, kind="ExternalOutput")

    with tile.TileContext(nc) as tc:
        with (
            tc.tile_pool(name="persist", bufs=1) as pp,
            tc.tile_pool(name="gep", bufs=4) as gepool,
            tc.tile_pool(name="stp", bufs=4) as stpool,
            tc.tile_pool(name="stw", bufs=4) as stwpool,
            tc.tile_pool(name="outp", bufs=3) as opool,
            tc.tile_pool(name="hpsum", bufs=8, space="PSUM") as hpsum,
        ):
            beta_b = pp.tile([128, 1], f32, tag="beta_b")
            nc.sync.dma_start(out=beta_b[:], in_=beta_ext[:])
            bl64 = pp.tile([128, 1], f32, tag="bl64")
            nc.vector.memset(bl64[:], -LN64)

            # (1+eps)*feat rows, entire shard resident
            o0 = pp.tile([128, HTILES * D], bf16, tag="o0")
            nc.sync.dma_start(out=o0[:], in_=o0_ext[:])

            for g in range(NG):
                i0 = g * GRP
                tiles = list(range(i0, min(i0 + GRP, HTILES)))
                gnh = int(gsum[g])

                straw = stpool.tile([128, GMAX, 128], f8, tag="straw")
                nc.sync.dma_start(
                    out=straw[:, :gnh, :].rearrange("p a b -> p (a b)"),
                    in_=st_ext[g * 128 : (g + 1) * 128, : gnh * 128],
                )
                ge = gepool.tile([128, GMAX, D], f8, tag="ge")
                nc.sync.dma_start(
                    out=ge[:, :gnh, :].rearrange("p a b -> p (a b)"),
                    in_=ge_ext[g * 128 : (g + 1) * 128, : gnh * D],
                )

                # stw = exp(beta*straw - ln64): w_e/64 one-hot-placed
                stw = stwpool.tile([128, GMAX, 128], bf16, tag="stw")
                nc.scalar.activation(
                    stw[:, :gnh, :].rearrange("p a b -> p (a b)"),
                    straw[:, :gnh, :].rearrange("p a b -> p (a b)"),
                    AF.Exp,
                    bias=bl64[:],
                    scale=beta_b[:],
                )

                og = opool.tile([128, GRP, D], bf16, tag="og")
                for i in tiles:
                    nh, lo = int(net[i]), int(loff[i])
                    hp = hpsum.tile([128, D], f32, tag="hp")
                    for t in range(nh):
                        nc.tensor.matmul(
                            hp[:],
                            stw[:, lo + t, :],
                            ge[:, lo + t, :],
                            start=(t == 0),
                            stop=(t == nh - 1),
                        )
                    nc.vector.tensor_tensor(
                        og[:, i - i0, :], o0[:, i * D : (i + 1) * D], hp[:],
                        ALU.add,
                    )
                nw = len(tiles)
                nc.sync.dma_start(
                    out=out_ext[:, i0 * D : (i0 + nw) * D],
                    in_=og[:, :nw, :].rearrange("p a b -> p (a b)"),
                )

    nc.finalize()
    return nc


def kernel(feat, edge_weight, src, dst, beta, eps):
    from concourse.bass_utils import run_bass_kernel_spmd
    import ml_dtypes

    feat = np.asarray(feat, dtype=np.float32)
    ew = np.asarray(edge_weight, dtype=np.float32)
    beta = np.asarray(beta, dtype=np.float32)
    eps = np.asarray(eps, dtype=np.float32)

    zrows, core_idx, net, K, loff, gsum, GMAX = _host_prep(src, dst, ew)

    key = (K, GMAX, tuple(int(x) for x in net))
    if key not in _COMPILED:
        _COMPILED[key] = (
            _build_phase1(K),
            _build_phase2(net, loff, gsum, GMAX),
        )
    nc1, nc2 = _COMPILED[key]

    beta128 = np.ascontiguousarray(np.broadcast_to(beta.reshape(1, 1), (128, 1)))
    eps128 = np.ascontiguousarray(np.broadcast_to(eps.reshape(1, 1), (128, 1)))

    # ---------------- phase 1: per-node g64, o0 ----------------
    in1 = []
    for c in range(NCORES):
        fmp = np.zeros((SHP, D), np.float32)
        fmp[:SH] = feat[c * SH : (c + 1) * SH]
        fmt = np.ascontiguousarray(
            fmp.reshape(HTILES, 128, D).transpose(1, 0, 2)
        ).reshape(128, HTILES * D).astype(ml_dtypes.bfloat16)
        in1.append(
            {"feat_my": fmt, "zrow": zrows[c], "beta128": beta128,
             "eps128": eps128}
        )

    res1 = run_bass_kernel_spmd(nc1, in1, core_ids=list(range(NCORES)))
    gfull = np.empty((N + 1, D), dtype=ml_dtypes.float8_e4m3)
    o0s = []
    for c in range(NCORES):
        gc = np.asarray(res1.results[c]["g64"]).reshape(128, HTILES, D)
        gfull[c * SH : (c + 1) * SH] = (
            gc.transpose(1, 0, 2).reshape(SHP, D)[:SH].astype(ml_dtypes.float8_e4m3)
        )
        o0s.append(np.asarray(res1.results[c]["o0"]))
    gfull[N] = 0  # pad row

    # ---------------- host gather of g64[src_e] ----------------
    in2 = []
    for c in range(NCORES):
        src_pad, straw = core_idx[c]
        ge = np.ascontiguousarray(
            gfull[src_pad].reshape(NG, GMAX, 128, D).transpose(0, 2, 1, 3)
        ).reshape(NG * 128, GMAX * D)
        in2.append(
            {"ge": ge, "straw": straw, "o0": o0s[c], "beta128": beta128}
        )

    res2 = run_bass_kernel_spmd(nc2, in2, core_ids=list(range(NCORES)))
    out = np.empty((N, D), np.float32)
    for c in range(NCORES):
        oc = np.asarray(res2.results[c]["out"]).reshape(128, HTILES, D)
        out[c * SH : (c + 1) * SH] = (
            oc.transpose(1, 0, 2).reshape(SHP, D)[:SH].astype(np.float32)
        )
    return out
